# revision 6
# baseline (speedup 1.0000x reference)
"""EuclideanCodebook (VQ) kernel for 8 Trainium2 NeuronCores.

Strategy (data-parallel, mirrors sharding_hint):
  - x [8, 32768, 128] sharded along tokens: core c takes batch c (32768 tokens).
  - codebook embed [1024, 128] replicated, passed pre-transposed + bf16 hi/lo split.
  - Scores s[n,k] = f.e - e2/2 via bf16 split matmuls (hi*ehi + hi*elo + lo*ehi)
    accumulated in fp32 PSUM, plus a rank-3 rider matmul adding an exact 3-way
    bf16 decomposition of -e2/2. Max + argmax via DVE Max8/MaxIndex from PSUM.
  - Device also returns the top-2 score values; the host re-checks the few
    tokens whose top-2 gap is below the split-error bound in fp64, making the
    final argmax exact. Host then gathers quantize and does the tiny EMA update.
"""
import sys
sys.path.insert(0, "/opt/trn_rl_repo")

import numpy as np
import ml_dtypes
from contextlib import ExitStack

import concourse.bass as bass
import concourse.tile as tile
from concourse import bacc, mybir
from concourse import bass_utils

N_CORES = 8
D = 128
K = 1024
N_PER_CORE = 32768
TOK_TILE = 128
N_TILES = N_PER_CORE // TOK_TILE  # 256
DECAY = 0.8
EPS = 1e-7
GAP_THRESHOLD = 2e-3  # > 2x the measured bf16-split score error bound

FP32 = mybir.dt.float32
BF16 = mybir.dt.bfloat16
U16 = mybir.dt.uint16
BF = ml_dtypes.bfloat16


def build_module(n_tiles: int):
    nc = bacc.Bacc("TRN2", target_bir_lowering=False, debug=False,
                   enable_asserts=False, num_devices=N_CORES)
    n_tok = n_tiles * TOK_TILE
    xh_d = nc.dram_tensor("xh", [D, n_tok], BF16, kind="ExternalInput").ap()
    xl_d = nc.dram_tensor("xl", [D, n_tok], BF16, kind="ExternalInput").ap()
    eh_d = nc.dram_tensor("eh", [D, K], BF16, kind="ExternalInput").ap()
    el_d = nc.dram_tensor("el", [D, K], BF16, kind="ExternalInput").ap()
    e2p_d = nc.dram_tensor("e2p", [3, K], BF16, kind="ExternalInput").ap()
    ind_d = nc.dram_tensor("ind", [128, n_tiles], U16, kind="ExternalOutput").ap()
    m2_d = nc.dram_tensor("m2", [128, 2 * n_tiles], FP32, kind="ExternalOutput").ap()

    with tile.TileContext(nc) as tc:
        with ExitStack() as ctx:
            const = ctx.enter_context(tc.tile_pool(name="const", bufs=1))
            xin = ctx.enter_context(tc.tile_pool(name="xin", bufs=6))
            fep = ctx.enter_context(tc.tile_pool(name="fep", bufs=3, space="PSUM"))
            small = ctx.enter_context(tc.tile_pool(name="small", bufs=6))

            eh = const.tile([D, K], BF16)
            nc.sync.dma_start(eh[:], eh_d[:])
            el = const.tile([D, K], BF16)
            nc.sync.dma_start(el[:], el_d[:])
            e2p = const.tile([3, K], BF16)
            nc.sync.dma_start(e2p[:], e2p_d[:])
            ones3 = const.tile([3, 128], BF16)
            nc.vector.memset(ones3[:], 1.0)
            idx_acc = const.tile([128, n_tiles], U16)
            m2_acc = const.tile([128, 2 * n_tiles], FP32)

            for i in range(n_tiles):
                sl = slice(i * TOK_TILE, (i + 1) * TOK_TILE)
                xh = xin.tile([D, TOK_TILE], BF16, tag="xh")
                nc.sync.dma_start(xh[:], xh_d[:, sl])
                xl = xin.tile([D, TOK_TILE], BF16, tag="xl")
                nc.sync.dma_start(xl[:], xl_d[:, sl])
                # s[tok, k] = xh.T@eh + xh.T@el + xl.T@eh + ones3.T@(-e2/2 parts)
                s = fep.tile([TOK_TILE, K], FP32)
                for h in (0, 1):
                    c = slice(h * 512, (h + 1) * 512)
                    nc.tensor.matmul(s[:, c], xh[:], eh[:, c], start=True, stop=False)
                    nc.tensor.matmul(s[:, c], xh[:], el[:, c], start=False, stop=False)
                    nc.tensor.matmul(s[:, c], xl[:], eh[:, c], start=False, stop=False)
                    nc.tensor.matmul(s[:, c], ones3[:], e2p[:, c], start=False, stop=True)
                # top-8 values + first-occurrence indices, straight from PSUM
                max8 = small.tile([TOK_TILE, 8], FP32)
                nc.vector.max(max8[:], s[:])
                idx8 = small.tile([TOK_TILE, 8], U16)
                nc.vector.max_index(idx8[:], max8[:], s[:])
                nc.vector.tensor_copy(idx_acc[:, i:i + 1], idx8[:, 0:1])
                nc.vector.tensor_copy(m2_acc[:, 2 * i:2 * i + 2], max8[:, 0:2])

            nc.sync.dma_start(ind_d[:], idx_acc[:])
            nc.sync.dma_start(m2_d[:], m2_acc[:])
    nc.compile()
    return nc


_NC_CACHE = {}


def _get_module(n_tiles=N_TILES):
    if n_tiles not in _NC_CACHE:
        _NC_CACHE[n_tiles] = build_module(n_tiles)
    return _NC_CACHE[n_tiles]


def _host_inputs(x, embed):
    flatten = np.ascontiguousarray(x.reshape(-1, D).astype(np.float32))
    xT = flatten.T  # [D, N] view
    xh = xT.astype(BF)
    xl = (xT - xh.astype(np.float32)).astype(BF)
    xh = np.ascontiguousarray(xh)
    xl = np.ascontiguousarray(xl)
    e32 = embed.astype(np.float32)
    eT = e32.T
    eh = np.ascontiguousarray(eT.astype(BF))
    el = np.ascontiguousarray((eT - eh.astype(np.float32)).astype(BF))
    e2 = (e32 * e32).sum(axis=1)
    v = -0.5 * e2
    p1 = v.astype(BF)
    p2 = (v - p1.astype(np.float32)).astype(BF)
    p3 = (v - p1.astype(np.float32) - p2.astype(np.float32)).astype(BF)
    e2p = np.ascontiguousarray(np.stack([p1, p2, p3], axis=0))
    return flatten, xh, xl, eh, el, e2p


def kernel(x, embed, ema_embed, ema_num):
    x = np.asarray(x)
    embed = np.asarray(embed)
    ema_embed = np.asarray(ema_embed)
    ema_num = np.asarray(ema_num)
    nc = _get_module()
    flatten, xh, xl, eh, el, e2p = _host_inputs(x, embed)
    in_maps = []
    for c in range(N_CORES):
        sl = slice(c * N_PER_CORE, (c + 1) * N_PER_CORE)
        in_maps.append({"xh": np.ascontiguousarray(xh[:, sl]),
                        "xl": np.ascontiguousarray(xl[:, sl]),
                        "eh": eh, "el": el, "e2p": e2p})
    res = bass_utils.run_bass_kernel_spmd(nc, in_maps, core_ids=list(range(N_CORES)))
    # gather/unshard
    ind = np.concatenate(
        [r["ind"].T.reshape(-1) for r in res.results]).astype(np.int64)   # [N]
    m2 = np.concatenate(
        [r["m2"].reshape(128, N_TILES, 2).transpose(1, 0, 2).reshape(-1, 2)
         for r in res.results])                                           # [N, 2]
    # exact fixup: re-evaluate tokens whose top-2 gap is below the error bound
    gap = m2[:, 0] - m2[:, 1]
    risky = np.where(gap < GAP_THRESHOLD)[0]
    if risky.size:
        e64 = embed.astype(np.float64)
        e2_64 = (e64 * e64).sum(axis=1)
        s64 = flatten[risky].astype(np.float64) @ e64.T - 0.5 * e2_64[None, :]
        ind[risky] = s64.argmax(axis=1)
    quantize = embed[ind].reshape(x.shape)
    counts = np.bincount(ind, minlength=K).astype(np.float32)
    sums = np.zeros((K, D), np.float32)
    np.add.at(sums, ind, flatten)
    ema_num_new = DECAY * ema_num + (1.0 - DECAY) * counts
    ema_embed_new = DECAY * ema_embed + (1.0 - DECAY) * sums
    total = ema_num_new.sum()
    smoothed = (ema_num_new + EPS) / (total + K * EPS) * total
    embed_new = ema_embed_new / smoothed[:, None]
    return quantize, embed_new, ema_num_new, ema_embed_new
